# revision 1
# baseline (speedup 1.0000x reference)
"""Trainium2 Bass kernel for nn_CVRP_Decoder (AFT-style attention-free decoder layer).

Data-parallel over batch: B=32 sharded as 4 batch elements per NeuronCore x 8 cores.
Host-side prep (layout only): per-batch transpose of scale_pairwise_dist (so the
contraction index lands on SBUF partitions) + bf16 cast of matmul operands.

Per-batch on-device pipeline (all token-major, tokens tiled 8 x 125):
  qkv   = dataT.T @ [Wq|Wk|Wv]            (PE, bf16)
  wT    = exp(alpha * distT)              (ACT, in-place on the DMA'd tile)
  ekv/ek tiles [j,256] = [exp(k)*v | exp(k)]
  num|den = accum_j wT[j,i].T @ ekv_ek[j] (PE, stationary = wT tile)
  aafm  = sigmoid(q) * num * recip(den)   (ACT+DVE)
  x1    = data + aafm; LN1 (DVE fused reduces + ACT sqrt)
  out1T = PE-transpose(out1) -> bf16      (ACT copy from PSUM)
  hT    = relu(W1.T @ out1T + b1)         (PE + ACT bias'd relu, feature-major)
  ff    = hT.T @ W2 + b2                  (PE, b2 via K=1 matmul)
  x2    = out1 + ff; LN2 -> out
"""

import os
import sys

import numpy as np

for _p in ("/opt/trn_rl_repo",):
    if _p not in sys.path and os.path.isdir(_p):
        sys.path.insert(0, _p)

import ml_dtypes
from contextlib import ExitStack

import concourse.bass as bass
import concourse.tile as tile
from concourse import bacc, mybir
from concourse import bass_utils

BF16 = ml_dtypes.bfloat16

B, N, D, F = 32, 1000, 128, 512
NCORES = 8
BPC = B // NCORES  # 4 batch elements per core
P = 128            # SBUF partitions
TS = 125           # token tile size (1000 = 8 * 125)
NT = N // TS       # 8 tiles
NFT = F // P       # 4 f-tiles
EPS = 1e-5

f32 = mybir.dt.float32
bf16 = mybir.dt.bfloat16


def _build(affine: bool):
    """Build + compile the per-core Bass module. `affine` includes the LN
    gamma/beta elementwise applications (skipped when they are identity)."""
    AF = mybir.ActivationFunctionType
    OP = mybir.AluOpType

    nc = bacc.Bacc("TRN2", target_bir_lowering=False, debug=False)

    # The act-table-load pass picks the first table set containing each
    # activation function, which splits Exp and Ln across two sets and incurs
    # a ~2.7us table switch per transition. All our ScalarE functions
    # (Exp, Ln, Copy, Relu) coexist in natural_log_exp_and_others, so steer
    # the selector there by removing Exp/Ln from every other set (in the
    # cached table dict; set ids are positional and unaffected).
    from concourse.hw_specs import get_activation_tables

    tabs = get_activation_tables(nc.m.arch)
    for name, funcs in tabs.items():
        if name != "natural_log_exp_and_others":
            funcs.discard(AF.Exp)
            funcs.discard(AF.Ln)

    # distT/data/out are host-pre-arranged into the exact SBUF layout so the
    # DMAs are plain 2D contiguous-per-partition transfers (few descriptors).
    distT = nc.dram_tensor("distT", (BPC, TS, NT * N), bf16, kind="ExternalInput").ap()
    dataT = nc.dram_tensor("dataT", (BPC, D, N), bf16, kind="ExternalInput").ap()
    data = nc.dram_tensor("data", (BPC, TS, NT * D), f32, kind="ExternalInput").ap()
    wqkv = nc.dram_tensor("wqkv", (D, 3 * D), bf16, kind="ExternalInput").ap()
    w1 = nc.dram_tensor("w1", (D, F), bf16, kind="ExternalInput").ap()
    w2r = nc.dram_tensor("w2r", (P, F), bf16, kind="ExternalInput").ap()
    b1c = nc.dram_tensor("b1c", (P, NFT), f32, kind="ExternalInput").ap()
    alpha = nc.dram_tensor("alpha", (P, 1), f32, kind="ExternalInput").ap()
    ident = nc.dram_tensor("ident", (P, P), f32, kind="ExternalInput").ap()
    if affine:
        g1t = nc.dram_tensor("g1t", (P, D), f32, kind="ExternalInput").ap()
        b1t = nc.dram_tensor("b1t", (P, D), f32, kind="ExternalInput").ap()
        g2t = nc.dram_tensor("g2t", (P, D), f32, kind="ExternalInput").ap()
        b2t = nc.dram_tensor("b2t", (P, D), f32, kind="ExternalInput").ap()
        # b2 enters the FF2 accumulation as ones128.T @ (b2/128 replicated)
        onesd = nc.dram_tensor("onesd", (P, TS), bf16, kind="ExternalInput").ap()
        b2rep = nc.dram_tensor("b2rep", (P, D), bf16, kind="ExternalInput").ap()
    out = nc.dram_tensor("out", (BPC, TS, NT * D), f32, kind="ExternalOutput").ap()

    with tile.TileContext(nc) as tc, ExitStack() as ctx:
        consts = ctx.enter_context(tc.tile_pool(name="consts", bufs=1))
        wqkv_sb = consts.tile([P, 3 * D], bf16)
        nc.sync.dma_start(wqkv_sb[:], wqkv)
        w1_sb = consts.tile([P, F], bf16)
        nc.sync.dma_start(w1_sb[:], w1)
        w2r_sb = consts.tile([P, F], bf16)
        nc.sync.dma_start(w2r_sb[:], w2r)
        b1c_sb = consts.tile([P, NFT], f32)
        nc.sync.dma_start(b1c_sb[:], b1c)
        alpha_sb = consts.tile([P, 1], f32)
        nc.sync.dma_start(alpha_sb[:], alpha)
        ident_sb = consts.tile([P, P], f32)
        nc.sync.dma_start(ident_sb[:], ident)
        eps_sb = consts.tile([P, 1], f32)
        nc.vector.memset(eps_sb[:], EPS)
        if affine:
            g1t_sb = consts.tile([P, D], f32)
            nc.sync.dma_start(g1t_sb[:], g1t)
            b1t_sb = consts.tile([P, D], f32)
            nc.sync.dma_start(b1t_sb[:], b1t)
            g2t_sb = consts.tile([P, D], f32)
            nc.sync.dma_start(g2t_sb[:], g2t)
            b2t_sb = consts.tile([P, D], f32)
            nc.sync.dma_start(b2t_sb[:], b2t)
            ones_sb = consts.tile([P, TS], bf16)
            nc.sync.dma_start(ones_sb[:], onesd)
            b2rep_sb = consts.tile([P, D], bf16)
            nc.sync.dma_start(b2rep_sb[:], b2rep)

        wT_pool = ctx.enter_context(tc.tile_pool(name="wT", bufs=3))
        dT_pool = ctx.enter_context(tc.tile_pool(name="dT", bufs=3))
        data_pool = ctx.enter_context(tc.tile_pool(name="datap", bufs=3))
        sq_pool = ctx.enter_context(tc.tile_pool(name="sq", bufs=3))
        ekv_pool = ctx.enter_context(tc.tile_pool(name="ekv", bufs=3))
        rcp_pool = ctx.enter_context(tc.tile_pool(name="rcp", bufs=2))
        r2_pool = ctx.enter_context(tc.tile_pool(name="r2", bufs=2))
        scr_pool = ctx.enter_context(tc.tile_pool(name="scr", bufs=2))
        x1_pool = ctx.enter_context(tc.tile_pool(name="x1", bufs=2))
        out1_pool = ctx.enter_context(tc.tile_pool(name="out1", bufs=2))
        o1T_pool = ctx.enter_context(tc.tile_pool(name="o1T", bufs=3))
        hT_pool = ctx.enter_context(tc.tile_pool(name="hT", bufs=2))
        x2_pool = ctx.enter_context(tc.tile_pool(name="x2", bufs=2))
        out3_pool = ctx.enter_context(tc.tile_pool(name="out3", bufs=2))
        st_pool = ctx.enter_context(tc.tile_pool(name="st", bufs=2))

        qkv_psum = ctx.enter_context(tc.tile_pool(name="qkvp", bufs=2, space="PSUM"))
        nd_psum = ctx.enter_context(tc.tile_pool(name="ndp", bufs=2, space="PSUM"))
        tr_psum = ctx.enter_context(tc.tile_pool(name="trp", bufs=1, space="PSUM"))
        hT_psum = ctx.enter_context(tc.tile_pool(name="hTp", bufs=1, space="PSUM"))
        ff_psum = ctx.enter_context(tc.tile_pool(name="ffp", bufs=1, space="PSUM"))

        for b in range(BPC):
            # ---- loads ----
            wT_t = wT_pool.tile([P, NT * N], bf16)
            # 8 SWDGE DMAs spread the 2MB load across DMA queues; per-j-tile
            # exp ops let the scheduler interleave other ACT work
            for j in range(NT):
                nc.gpsimd.dma_start(
                    wT_t[0:TS, j * N : (j + 1) * N], distT[b][:, j * N : (j + 1) * N]
                )
                nc.scalar.activation(
                    wT_t[0:TS, j * N : (j + 1) * N],
                    wT_t[0:TS, j * N : (j + 1) * N],
                    AF.Exp, scale=alpha_sb[0:TS, 0:1],
                )

            dataT_sb = dT_pool.tile([P, N], bf16)
            nc.scalar.dma_start(dataT_sb[:], dataT[b])
            data_sb = data_pool.tile([P, NT * D], f32)
            nc.scalar.dma_start(data_sb[0:TS, :], data[b])

            # ---- qkv + ekv/ek ----
            # sigmoid(q) is realized as 1/(1+exp(-q)) folded into the AFT
            # denominator: aafm = num * recip((1+exp(-q)) * den). This keeps
            # ScalarE on a single activation-table set (natural_log_exp).
            eq_sb = sq_pool.tile([P, NT * D], bf16)
            ekv_sb = ekv_pool.tile([P, NT * 2 * D], bf16)
            for t in range(NT):
                qkv_ps = qkv_psum.tile([P, 3 * D], f32)
                nc.tensor.matmul(
                    qkv_ps[0:TS, :],
                    dataT_sb[:, t * TS : (t + 1) * TS],
                    wqkv_sb[:],
                    start=True,
                    stop=True,
                )
                nc.scalar.activation(
                    eq_sb[0:TS, t * D : (t + 1) * D], qkv_ps[0:TS, 0:D], AF.Exp,
                    scale=-1.0,
                )
                nc.scalar.activation(
                    ekv_sb[0:TS, t * 2 * D + D : (t + 1) * 2 * D],
                    qkv_ps[0:TS, D : 2 * D],
                    AF.Exp,
                )
                nc.vector.tensor_tensor(
                    ekv_sb[0:TS, t * 2 * D : t * 2 * D + D],
                    qkv_ps[0:TS, 2 * D : 3 * D],
                    ekv_sb[0:TS, t * 2 * D + D : (t + 1) * 2 * D],
                    OP.mult,
                )

            # ---- AFT num/den + combine + residual1 ----
            x1_sb = x1_pool.tile([P, NT * D], f32)
            rcp_sb = rcp_pool.tile([P, NT * D], f32)
            r2_sb = r2_pool.tile([P, NT * D], f32)
            scr_sb = scr_pool.tile([P, NT * D], f32)
            for t in range(NT):
                nd_ps = nd_psum.tile([P, 2 * D], f32)
                for j in range(NT):
                    nc.tensor.matmul(
                        nd_ps[0:TS, :],
                        wT_t[0:TS, j * N + t * TS : j * N + (t + 1) * TS],
                        ekv_sb[0:TS, j * 2 * D : (j + 1) * 2 * D],
                        start=(j == 0),
                        stop=(j == NT - 1),
                    )
                # u = (exp(-q) + 1) * den ; aafm = num * recip(u)
                u = rcp_sb[0:TS, t * D : (t + 1) * D]
                nc.vector.scalar_tensor_tensor(
                    u, eq_sb[0:TS, t * D : (t + 1) * D], 1.0,
                    nd_ps[0:TS, D : 2 * D], OP.add, OP.mult,
                )
                r = r2_sb[0:TS, t * D : (t + 1) * D]
                nc.vector.reciprocal_approx_fast(out=r, in_=u)
                tmp = scr_sb[0:TS, t * D : (t + 1) * D]
                nc.vector.tensor_tensor(tmp, nd_ps[0:TS, 0:D], r, OP.mult)
                nc.vector.tensor_tensor(
                    x1_sb[0:TS, t * D : (t + 1) * D], tmp,
                    data_sb[0:TS, t * D : (t + 1) * D], OP.add,
                )

            # ---- LN1 (bn_stats -> mean/var per token tile) ----
            bn1 = st_pool.tile([P, NT * 6], f32)
            stats1 = st_pool.tile([P, NT * 2], f32)
            rstd1 = st_pool.tile([P, NT], f32)
            for t in range(NT):
                nc.vector.bn_stats(
                    bn1[0:TS, 6 * t : 6 * t + 6], x1_sb[0:TS, t * D : (t + 1) * D]
                )
                nc.vector.bn_aggr(
                    stats1[0:TS, 2 * t : 2 * t + 2], bn1[0:TS, 6 * t : 6 * t + 6]
                )
            # rstd = (var+eps)^-0.5 = exp(-0.5*ln(var+eps)); ln & exp share one ACT table set
            nc.scalar.activation(
                rstd1[0:TS, :],
                stats1[0:TS, :].rearrange("p (t s) -> p t s", s=2)[:, :, 1],
                AF.Ln, bias=eps_sb[0:TS, 0:1],
            )
            nc.scalar.activation(rstd1[0:TS, :], rstd1[0:TS, :], AF.Exp, scale=-0.5)

            out1_sb = out1_pool.tile([P, NT * D], f32)
            for t in range(NT):
                o1 = out1_sb[0:TS, t * D : (t + 1) * D]
                nc.vector.tensor_scalar(
                    o1, x1_sb[0:TS, t * D : (t + 1) * D],
                    stats1[0:TS, 2 * t : 2 * t + 1], rstd1[0:TS, t : t + 1],
                    OP.subtract, OP.mult,
                )
                if affine:
                    nc.vector.tensor_tensor(o1, o1, g1t_sb[0:TS, :], OP.mult)
                    nc.vector.tensor_tensor(o1, o1, b1t_sb[0:TS, :], OP.add)

            # ---- transpose out1 -> out1T (bf16) ----
            out1T_sb = o1T_pool.tile([P, N], bf16)
            for t in range(NT):
                tr_ps = tr_psum.tile([P, TS], f32)
                nc.tensor.transpose(
                    tr_ps[:, 0:TS],
                    out1_sb[0:TS, t * D : (t + 1) * D],
                    ident_sb[0:TS, 0:TS],
                )
                nc.vector.tensor_copy(out1T_sb[:, t * TS : (t + 1) * TS], tr_ps[:, 0:TS])

            # ---- FF1: hT = relu(W1.T @ out1T + b1), feature-major ----
            hT_sb = hT_pool.tile([P, NFT * N], bf16)
            for ft in range(NFT):
                hT_ps = hT_psum.tile([P, 1024], f32)
                nc.tensor.matmul(
                    hT_ps[:, 0:512], w1_sb[:, ft * P : (ft + 1) * P],
                    out1T_sb[:, 0:512], start=True, stop=True,
                )
                nc.tensor.matmul(
                    hT_ps[:, 512:1000], w1_sb[:, ft * P : (ft + 1) * P],
                    out1T_sb[:, 512:1000], start=True, stop=True,
                )
                nc.scalar.activation(
                    hT_sb[:, ft * N : ft * N + 512], hT_ps[:, 0:512],
                    AF.Relu, bias=b1c_sb[:, ft : ft + 1],
                )
                nc.scalar.activation(
                    hT_sb[:, ft * N + 512 : (ft + 1) * N], hT_ps[:, 512:1000],
                    AF.Relu, bias=b1c_sb[:, ft : ft + 1],
                )

            # ---- FF2 + residual2 ----
            x2_sb = x2_pool.tile([P, NT * D], f32)
            for t in range(NT):
                ff_ps = ff_psum.tile([P, D], f32)
                for ft in range(NFT):
                    nc.tensor.matmul(
                        ff_ps[0:TS, :],
                        hT_sb[:, ft * N + t * TS : ft * N + (t + 1) * TS],
                        w2r_sb[:, ft * D : (ft + 1) * D],
                        start=(ft == 0), stop=(ft == NFT - 1 and not affine),
                    )
                if affine:
                    nc.tensor.matmul(
                        ff_ps[0:TS, :], ones_sb[:, 0:TS], b2rep_sb[:, :],
                        start=False, stop=True,
                    )
                nc.vector.tensor_tensor(
                    x2_sb[0:TS, t * D : (t + 1) * D], ff_ps[0:TS, :],
                    out1_sb[0:TS, t * D : (t + 1) * D], OP.add,
                )

            # ---- LN2 -> out ----
            bn2 = st_pool.tile([P, NT * 6], f32)
            stats2 = st_pool.tile([P, NT * 2], f32)
            rstd2 = st_pool.tile([P, NT], f32)
            for t in range(NT):
                nc.vector.bn_stats(
                    bn2[0:TS, 6 * t : 6 * t + 6], x2_sb[0:TS, t * D : (t + 1) * D]
                )
                nc.vector.bn_aggr(
                    stats2[0:TS, 2 * t : 2 * t + 2], bn2[0:TS, 6 * t : 6 * t + 6]
                )
            nc.scalar.activation(
                rstd2[0:TS, :],
                stats2[0:TS, :].rearrange("p (t s) -> p t s", s=2)[:, :, 1],
                AF.Ln, bias=eps_sb[0:TS, 0:1],
            )
            nc.scalar.activation(rstd2[0:TS, :], rstd2[0:TS, :], AF.Exp, scale=-0.5)

            out3_sb = out3_pool.tile([P, NT * D], f32)
            for t in range(NT):
                o3 = out3_sb[0:TS, t * D : (t + 1) * D]
                nc.vector.tensor_scalar(
                    o3, x2_sb[0:TS, t * D : (t + 1) * D],
                    stats2[0:TS, 2 * t : 2 * t + 1], rstd2[0:TS, t : t + 1],
                    OP.subtract, OP.mult,
                )
                if affine:
                    nc.vector.tensor_tensor(o3, o3, g2t_sb[0:TS, :], OP.mult)
                    nc.vector.tensor_tensor(o3, o3, b2t_sb[0:TS, :], OP.add)

            nc.scalar.dma_start(out[b], out3_sb[0:TS, :])

    nc.compile()
    return nc


_CACHE: dict = {}


def _get_module(affine: bool):
    if affine not in _CACHE:
        _CACHE[affine] = _build(affine)
    return _CACHE[affine]


TRACE = False
LAST_RESULTS = None


def kernel(**inputs) -> np.ndarray:
    data = np.ascontiguousarray(np.asarray(inputs["data"], dtype=np.float32))
    dist = np.asarray(inputs["scale_pairwise_dist"], dtype=np.float32)
    mask = np.asarray(inputs["ninf_mask"], dtype=np.float32)
    Wq = np.asarray(inputs["Wq"], dtype=np.float32)
    Wk = np.asarray(inputs["Wk"], dtype=np.float32)
    Wv = np.asarray(inputs["Wv"], dtype=np.float32)
    alpha_attn = np.asarray(inputs["alpha_attn"], dtype=np.float32)
    ln1_g = np.asarray(inputs["ln1_g"], dtype=np.float32)
    ln1_b = np.asarray(inputs["ln1_b"], dtype=np.float32)
    ln2_g = np.asarray(inputs["ln2_g"], dtype=np.float32)
    ln2_b = np.asarray(inputs["ln2_b"], dtype=np.float32)
    W1 = np.asarray(inputs["W1"], dtype=np.float32)
    b1 = np.asarray(inputs["b1"], dtype=np.float32)
    W2 = np.asarray(inputs["W2"], dtype=np.float32)
    b2 = np.asarray(inputs["b2"], dtype=np.float32)

    affine = not (
        np.all(ln1_g == 1.0) and np.all(ln1_b == 0.0)
        and np.all(ln2_g == 1.0) and np.all(ln2_b == 0.0)
        and np.all(b2 == 0.0)
    )
    mask_nonzero = bool(np.any(mask != 0.0))

    nc = _get_module(affine)

    # shared (replicated) small tensors
    wqkv_np = np.concatenate([Wq, Wk, Wv], axis=1).astype(BF16)
    w1_np = W1.astype(BF16)
    w2r_np = np.ascontiguousarray(
        W2.reshape(NFT, P, D).transpose(1, 0, 2)
    ).reshape(P, NFT * D).astype(BF16)
    b1c_np = np.ascontiguousarray(b1.reshape(NFT, P).T).astype(np.float32)
    ident_np = np.eye(P, dtype=np.float32)
    common = {
        "wqkv": wqkv_np, "w1": w1_np, "w2r": w2r_np, "b1c": b1c_np,
        "ident": ident_np,
    }
    if affine:
        common["g1t"] = np.tile(ln1_g.reshape(1, D), (P, 1)).astype(np.float32)
        common["b1t"] = np.tile(ln1_b.reshape(1, D), (P, 1)).astype(np.float32)
        common["g2t"] = np.tile(ln2_g.reshape(1, D), (P, 1)).astype(np.float32)
        common["b2t"] = np.tile(ln2_b.reshape(1, D), (P, 1)).astype(np.float32)
        common["onesd"] = np.ones((P, TS), dtype=BF16)
        common["b2rep"] = np.tile((b2 / P).reshape(1, D), (P, 1)).astype(BF16)

    if mask_nonzero:
        # fold mask in on host: exp(alpha*dist + mask) == exp(1.0 * (alpha*dist + mask))
        eff = alpha_attn[0] * dist + mask
        alpha_np = np.ones((P, 1), dtype=np.float32)
    else:
        eff = dist
        alpha_np = np.full((P, 1), alpha_attn[0], dtype=np.float32)
    common["alpha"] = alpha_np

    in_maps = []
    for c in range(NCORES):
        sl = slice(BPC * c, BPC * (c + 1))
        m = dict(common)
        # distT[b, p, j*N+i] = eff[b, i, j*TS+p] : SBUF layout [p, (j i)]
        m["distT"] = np.ascontiguousarray(
            eff[sl].transpose(0, 2, 1).reshape(BPC, NT, TS, N).transpose(0, 2, 1, 3)
        ).reshape(BPC, TS, NT * N).astype(BF16)
        m["dataT"] = data[sl].transpose(0, 2, 1).astype(BF16)
        # data[b, p, t*D+d] = data[b, t*TS+p, d] : SBUF layout [p, (t d)]
        m["data"] = np.ascontiguousarray(
            data[sl].reshape(BPC, NT, TS, D).transpose(0, 2, 1, 3)
        ).reshape(BPC, TS, NT * D)
        in_maps.append(m)

    res = bass_utils.run_bass_kernel_spmd(
        nc, in_maps, core_ids=list(range(NCORES)), trace=TRACE
    )
    global LAST_RESULTS
    LAST_RESULTS = res
    outs = []
    for c in range(NCORES):
        o = res.results[c]["out"].reshape(BPC, TS, NT, D).transpose(0, 2, 1, 3)
        outs.append(np.ascontiguousarray(o).reshape(BPC, N, D))
    return np.concatenate(outs, axis=0)

